# revision 1
# baseline (speedup 1.0000x reference)
"""Trainium2 Bass kernel for nn_CNNseq_15564961481149 (dense_cnn).

Computes: embed lookup -> 3 parallel 1-D convs (K=3,4,5, channels-first)
-> bias -> max-over-time -> concat -> relu, for text [16, 64, 128] over a
[30000, 512] embedding table, F=256 filters per conv.

Strategy (pure data parallel over 8 NeuronCores):
  - Flatten B*S = 1024 samples; 128 samples per core.
  - Embedding table converted to fp16 on host, gathered on-device with
    dma_gather(transpose=True): the gathered tile lands as
    [d%128 (partition), d//128 (chunk), token] -- exactly the moving-operand
    layout the PE needs (contraction dim on partitions).
  - Conv-as-matmul: for each tap j and d-chunk c, a [128d, 128f] stationary
    tile multiplies the token columns shifted by j; accumulated in PSUM over
    all (c, j).  4 samples per matmul via a 2-level free dim (4 x L_out <= 512).
  - max-over-time on DVE straight out of PSUM, bias+relu folded in after
    (max(y + b) == max(y) + b), PE-transpose of the [f, sample] result so the
    final DMA to DRAM is fully contiguous.

Inputs are fp16-quantized for the matmul (fp32 accumulate in PSUM); measured
end-to-end max elementwise relative error vs fp64 is ~7e-4.
"""

from contextlib import ExitStack

import numpy as np

import concourse.mybir as mybir
import concourse.tile as tile
from concourse import bacc
from concourse.bass_utils import run_bass_kernel_spmd
from concourse.masks import make_identity

# Problem constants (hardcoded per harness contract).
B, S, L, D, F, V = 16, 64, 128, 512, 256, 30000
N_CORES = 8
NSAMP = B * S // N_CORES          # 128 samples per core
KS = (3, 4, 5)                    # conv kernel sizes
SLOT_BASE = (0, 3, 7)             # tap-slot offsets for conv3/4/5 (12 total)
N_SLOTS = 12
SPG = 4                           # samples per gather tile (512 idxs; >512 crashes the SWDGE transpose-gather)
NGT = NSAMP // SPG                # gather tiles per core
GPT = SPG // 4                    # groups of 4 samples per gather tile

f16 = mybir.dt.float16
f32 = mybir.dt.float32
i16 = mybir.dt.int16


def build_nc(nsamp=NSAMP, spg=SPG, mode="full"):
    """Build the per-core Bass program (SPMD: same program, 8 cores).

    mode: "full" | "nogather" (memset x tiles) | "nomm" (skip matmul/reduce)
    -- the reduced modes exist only to attribute wall-clock time.
    """
    ngt = nsamp // spg
    gpt = spg // 4
    t_tot = nsamp * L                  # tokens per core
    n_idx = spg * L                    # tokens per gather
    idx_cols_per_gather = n_idx // 16

    nc = bacc.Bacc("TRN2", target_bir_lowering=False, debug=False,
                   num_devices=N_CORES)

    emb_h = nc.dram_tensor("emb", [V, D], f16, kind="ExternalInput")
    idx_h = nc.dram_tensor("idx", [128, t_tot // 16], i16, kind="ExternalInput")
    wst_h = nc.dram_tensor("wst", [128, N_SLOTS, 4, 2, 128], f16,
                           kind="ExternalInput")
    bias_h = nc.dram_tensor("bias", [128, 6], f32, kind="ExternalInput")
    out_h = nc.dram_tensor("out", [nsamp, 3 * F], f32, kind="ExternalOutput")

    with tile.TileContext(nc) as tc, ExitStack() as ctx:
        cpool = ctx.enter_context(tc.tile_pool(name="consts", bufs=1))
        xpool = ctx.enter_context(tc.tile_pool(name="x", bufs=6))
        pspool = ctx.enter_context(
            tc.tile_pool(name="ps", bufs=6, space="PSUM"))
        tppool = ctx.enter_context(
            tc.tile_pool(name="tp", bufs=2, space="PSUM"))

        idx_sb = cpool.tile([128, t_tot // 16], i16)
        w_sb = cpool.tile([128, N_SLOTS, 4, 2, 128], f16)
        bias_sb = cpool.tile([128, 6], f32)
        ident = cpool.tile([128, 128], f32)
        out_sb = cpool.tile([128, 6, nsamp], f32)
        out_t = cpool.tile([nsamp, 6 * 128], f32)

        nc.sync.dma_start(out=idx_sb[:], in_=idx_h.ap()[:])
        nc.sync.dma_start(out=w_sb[:], in_=wst_h.ap()[:])
        nc.sync.dma_start(out=bias_sb[:], in_=bias_h.ap()[:])
        make_identity(nc, ident[:])

        if mode == "nomm":
            nc.gpsimd.memset(out_sb[:], 0.0)
        reps = int(mode[len("repeat"):]) if mode.startswith("repeat") else 0
        loop_cm = tc.For_i(0, reps, 1) if reps else None
        if loop_cm is not None:
            loop_cm.__enter__()
        # Process gather tiles in batches of QB so each stationary weight tile
        # is reused across QB matmuls (amortizes LDWEIGHTS 4x).
        # QB>1 (stationary reuse across gather tiles) measured ~28% SLOWER on
        # HW than back-to-back same-bank matmuls: LDWEIGHTS is already hidden
        # by the PE reorder window + dual SBUF read ports, and interleaving
        # PSUM banks/operand buffers per instruction costs more than it saves.
        QB = 1
        for sup in range(ngt // QB):
            xvs = []
            for q in range(QB):
                t = sup * QB + q
                xt = xpool.tile([128, 4, n_idx], f16, tag="xt")
                if mode == "nogather":
                    nc.gpsimd.memset(xt[:], 0.0)
                else:
                    nc.gpsimd.dma_gather(
                        out_ap=xt[:],
                        in_ap=emb_h.ap()[:],
                        idxs_ap=idx_sb[:, t * idx_cols_per_gather:
                                       (t + 1) * idx_cols_per_gather],
                        num_idxs=n_idx,
                        num_idxs_reg=n_idx,
                        elem_size=D,
                        transpose=True,
                    )
                if mode == "nomm":
                    nc.vector.tensor_copy(out_sb[:, 0, t:t + 1], xt[:, 0, :1])
                xvs.append(xt.rearrange("p c (s l) -> p c s l", s=spg))
            if mode == "nomm":
                continue
            for k_idx, K in enumerate(KS):
                lout = L - K + 1
                for m in range(2):
                    pss = [pspool.tile([128, 4, lout], f32, tag="ps",
                                       name=f"ps_{sup}_{k_idx}_{m}_{q}")
                           for q in range(QB)]
                    n_mm = 4 * K
                    mm = 0
                    for c in range(4):
                        for j in range(K):
                            for q in range(QB):
                                nc.tensor.matmul(
                                    pss[q][:],
                                    w_sb[:, SLOT_BASE[k_idx] + j, c, m, :],
                                    xvs[q][:, c, 0:4, j:j + lout],
                                    start=(mm == 0),
                                    stop=(mm == n_mm - 1),
                                )
                            mm += 1
                    tile6 = k_idx * 2 + m
                    for q in range(QB):
                        gidx = sup * QB + q
                        nc.vector.reduce_max(
                            out_sb[:, tile6, gidx * 4:gidx * 4 + 4],
                            pss[q][:],
                            axis=mybir.AxisListType.X,
                        )

        if loop_cm is not None:
            loop_cm.__exit__(None, None, None)
        # bias + relu on [f(partition), sample] layout, then PE-transpose so
        # the final DMA writes contiguous [sample, 768] rows.
        for tile6 in range(6):
            nc.vector.tensor_scalar(
                out_sb[:, tile6, :], out_sb[:, tile6, :],
                bias_sb[:, tile6:tile6 + 1], 0.0,
                op0=mybir.AluOpType.add, op1=mybir.AluOpType.max,
            )
            tp = tppool.tile([nsamp, 128], f32, tag="tp")
            nc.tensor.transpose(tp[:], out_sb[:, tile6, :], ident[:])
            nc.vector.tensor_copy(
                out_t[:, tile6 * 128:(tile6 + 1) * 128], tp[:])
        nc.sync.dma_start(out=out_h.ap()[:], in_=out_t[:])

    nc.compile()
    return nc


def prep_inputs(text, embed, w3, b3, w4, b4, w5, b5, nsamp=NSAMP, spg=SPG,
                n_cores=N_CORES):
    """Host-side marshaling: shard text, wrap gather indices, fp16-quantize
    and retile the weights/embedding."""
    text = np.ascontiguousarray(np.asarray(text).reshape(B * S, L))
    assert text.max() < V and text.min() >= 0
    emb16 = np.ascontiguousarray(np.asarray(embed, dtype=np.float16))

    wst = np.zeros((128, N_SLOTS, 4, 2, 128), np.float16)
    for k_idx, w in enumerate((w3, w4, w5)):
        w = np.asarray(w, dtype=np.float32)
        for j in range(KS[k_idx]):
            # wst[dd, slot, c, m, ff] = w[m*128+ff, c*128+dd, j]
            wj = w[:, :, j].reshape(2, 128, 4, 128)      # [m, ff, c, dd]
            wst[:, SLOT_BASE[k_idx] + j] = wj.transpose(3, 2, 0, 1)
    wst = np.ascontiguousarray(wst)

    bias = np.zeros((128, 6), np.float32)
    for k_idx, b in enumerate((b3, b4, b5)):
        bias[:, 2 * k_idx:2 * k_idx + 2] = \
            np.asarray(b, dtype=np.float32).reshape(2, 128).T
    bias = np.ascontiguousarray(bias)

    ngt = nsamp // spg
    in_maps = []
    for r in range(n_cores):
        tcore = text[r * nsamp:(r + 1) * nsamp].astype(np.int16)
        # token i of gather tile t -> partition i%16, column t*(spg*L/16)+i//16;
        # the 16-row block must be replicated to all 128 partitions (each of
        # the 8 gpsimd sub-cores reads its own 16-partition stripe).
        a = tcore.reshape(ngt, spg * L // 16, 16)         # [t, c, p]
        idx = np.tile(a.transpose(2, 0, 1).reshape(16, -1), (8, 1))
        in_maps.append({
            "emb": emb16,
            "idx": np.ascontiguousarray(idx),
            "wst": wst,
            "bias": bias,
        })
    return in_maps


_CACHE = {}


def kernel(text, embed, w3, b3, w4, b4, w5, b5):
    if "nc" not in _CACHE:
        _CACHE["nc"] = build_nc()
    nc = _CACHE["nc"]
    in_maps = prep_inputs(text, embed, w3, b3, w4, b4, w5, b5)
    res = run_bass_kernel_spmd(nc, in_maps, list(range(N_CORES)))
    out = np.concatenate([res.results[r]["out"] for r in range(N_CORES)],
                         axis=0)
    return out.reshape(B, S, 3 * F).astype(np.float32)



# revision 32
# speedup vs baseline: 1.1130x; 1.1130x over previous
"""Trainium2 Bass kernel for nn_CNNseq_15564961481149 (dense_cnn).

Computes: embed lookup -> 3 parallel 1-D convs (K=3,4,5, channels-first)
-> bias -> max-over-time -> concat -> relu, for text [16, 64, 128] over a
[30000, 512] embedding table, F=256 filters per conv.

Strategy (pure data parallel over 8 NeuronCores):
  - Flatten B*S = 1024 samples; 128 samples per core.
  - Embedding table converted to bf16 on host, gathered on-device with
    dma_gather(transpose=True): the gathered tile lands as
    [d%128 (partition), d//128 (chunk), token] -- exactly the moving-operand
    layout the PE needs (contraction dim on partitions).  All 32 gather
    tiles (128 KB/partition) are SBUF-resident and issued up-front across
    4 SWDGE queues, so the matmul stream never recycles x buffers.
  - Conv-as-matmul: for each tap j and d-chunk c, a [128d, 128f] stationary
    tile multiplies the token columns shifted by j; accumulated in PSUM over
    all (c, j).  4 samples per matmul via a 2-level free dim (4 x L_out <= 512).
  - max-over-time on DVE straight out of PSUM, bias+relu folded in after
    (max(y + b) == max(y) + b), PE-transpose of the [f, sample] result so the
    final DMA to DRAM is fully contiguous.

Inputs are bf16-quantized for the matmul (fp32 accumulate in PSUM); measured
end-to-end max elementwise relative error is ~5.4e-3 (gate 2e-2).  bf16's
fp32-range exponent keeps N(0,1) embeddings out of the subnormal range and
its 7-bit-mantissa multipliers draw less PE power than fp16 (the kernel is
power/throttle-bound on real data: identical matmul streams run ~590us on
all-zero inputs vs ~850us on N(0,1) data at the 8/32-repeat measurement).
"""

from contextlib import ExitStack

import numpy as np

import concourse.mybir as mybir
import concourse.tile as tile
from concourse import bacc
from concourse.bass_utils import run_bass_kernel_spmd
from concourse.masks import make_identity

# Problem constants (hardcoded per harness contract).
B, S, L, D, F, V = 16, 64, 128, 512, 256, 30000
N_CORES = 8
NSAMP = B * S // N_CORES          # 128 samples per core
KS = (3, 4, 5)                    # conv kernel sizes
SLOT_BASE = (0, 3, 7)             # tap-slot offsets for conv3/4/5 (12 total)
N_SLOTS = 12
SPG = 4                           # samples per gather tile (512 idxs; >512 crashes the SWDGE transpose-gather)
NGT = NSAMP // SPG                # gather tiles per core
GPT = SPG // 4                    # groups of 4 samples per gather tile

f16 = mybir.dt.float16
bf16 = mybir.dt.bfloat16
f32 = mybir.dt.float32
i16 = mybir.dt.int16


def build_nc(nsamp=NSAMP, spg=SPG, mode="full"):
    """Build the per-core Bass program (SPMD: same program, 8 cores).

    mode: "[repeatN][:body]".  body: "full" | "nogather" (memset x tiles
    once, keep matmuls) | "gonly" (gathers only) | "samew" (all matmuls use
    one stationary tile) | "noreduce" (no reduce_max) -- reduced bodies
    exist only to attribute wall-clock time on HW.
    """
    if ":" in mode:
        rep_s, body = mode.split(":")
    elif mode.startswith("repeat"):
        rep_s, body = mode, "full"
    else:
        rep_s, body = "", mode
    reps = int(rep_s[len("repeat"):]) if rep_s.startswith("repeat") else 0
    parts = body.split("+")
    body, flags = parts[0], set(parts[1:])
    n_queues = 4 if "q4" in flags else 1
    single_pkt = "sp0" not in flags
    gather_stride = 2 if "g16" in flags else 1
    xb = 6
    for f in flags:
        if f.startswith("xb"):
            xb = int(f[2:])
    mdt = bf16 if "bf" in flags else f16

    ngt = nsamp // spg
    gpt = spg // 4
    t_tot = nsamp * L                  # tokens per core
    n_idx = spg * L                    # tokens per gather
    idx_cols_per_gather = n_idx // 16

    nc = bacc.Bacc("TRN2", target_bir_lowering=False, debug=False,
                   num_devices=N_CORES, num_swdge_queues=n_queues)

    emb_h = nc.dram_tensor("emb", [V, D], mdt, kind="ExternalInput")
    idx_h = nc.dram_tensor("idx", [128, t_tot // 16], i16, kind="ExternalInput")
    wst_h = nc.dram_tensor("wst", [128, N_SLOTS, 4, 2, 128], mdt,
                           kind="ExternalInput")
    bias_h = nc.dram_tensor("bias", [128, 6], f32, kind="ExternalInput")
    out_h = nc.dram_tensor("out", [nsamp, 3 * F], f32, kind="ExternalOutput")

    with tile.TileContext(nc) as tc, ExitStack() as ctx:
        cpool = ctx.enter_context(tc.tile_pool(name="consts", bufs=1))
        xpool = ctx.enter_context(tc.tile_pool(name="x", bufs=xb))
        pspool = ctx.enter_context(
            tc.tile_pool(name="ps", bufs=6, space="PSUM"))
        tppool = ctx.enter_context(
            tc.tile_pool(name="tp", bufs=2, space="PSUM"))

        idx_sb = cpool.tile([128, t_tot // 16], i16)
        w_sb = cpool.tile([128, N_SLOTS, 4, 2, 128], mdt)
        bias_sb = cpool.tile([128, 6], f32)
        ident = cpool.tile([128, 128], f32)
        out_sb = cpool.tile([128, 6, nsamp], f32)
        out_t = cpool.tile([nsamp, 6 * 128], f32)

        nc.sync.dma_start(out=idx_sb[:], in_=idx_h.ap()[:])
        nc.sync.dma_start(out=w_sb[:], in_=wst_h.ap()[:])
        nc.sync.dma_start(out=bias_sb[:], in_=bias_h.ap()[:])
        make_identity(nc, ident[:])

        if body in ("gonly", "noreduce", "mmonly"):
            nc.gpsimd.memset(out_sb[:], 0.0)
        XB = xb                         # x double-buffer depth
        xall = None
        if body in ("gall", "relay"):
            # whole per-core x resident in SBUF: 32 gathers up-front into one
            # tile; MM stream then has no mid-stream gather WAR recycling.
            xall = cpool.tile([128, ngt, 4, n_idx], mdt)
        if body in ("nogather", "mmonly", "gfree", "randx"):
            xts = [xpool.tile([128, 4, n_idx], mdt, tag=f"xg{i}",
                              name=f"xg{i}")
                   for i in range(XB)]
            if body == "randx":
                # real N(0,1) data via plain contiguous DMA (no gather):
                # discriminates data-dependent power from gather overhead.
                src = emb_h.ap()[:XB * 512].rearrange(
                    "(o p f) d -> o p (f d)", p=128, f=4)
                for i, x in enumerate(xts):
                    nc.sync.dma_start(out=x[:], in_=src[i])
            else:
                for x in xts:
                    nc.gpsimd.memset(x[:], 0.0)

        loop_cm = tc.For_i(0, reps, 1) if reps else None
        if loop_cm is not None:
            loop_cm.__enter__()
        # NOTE: QB>1 (stationary reuse across gather tiles) measured ~28%
        # SLOWER on HW than back-to-back same-bank matmuls.
        if body == "gall":
            for sup in range(ngt):
                nc.gpsimd.dma_gather(
                    out_ap=xall[:, sup],
                    in_ap=emb_h.ap()[:],
                    idxs_ap=idx_sb[:, sup * idx_cols_per_gather:
                                   (sup + 1) * idx_cols_per_gather],
                    num_idxs=n_idx,
                    num_idxs_reg=n_idx,
                    elem_size=D,
                    transpose=True,
                    queue_num=sup % n_queues,
                    single_packet=single_pkt,
                )
        elif body == "relay":
            # gather -> stage, then ScalarE copies stage -> xall: PE's data
            # dependency lands on ACT's engine semaphore (cheap) instead of a
            # DMA-completion semaphore (measured ~5us per PE-side wait).
            for sup in range(ngt):
                stg = xpool.tile([128, 4, n_idx], mdt, tag="stg", bufs=4,
                                 name=f"stg{sup}")
                nc.gpsimd.dma_gather(
                    out_ap=stg[:],
                    in_ap=emb_h.ap()[:],
                    idxs_ap=idx_sb[:, sup * idx_cols_per_gather:
                                   (sup + 1) * idx_cols_per_gather],
                    num_idxs=n_idx,
                    num_idxs_reg=n_idx,
                    elem_size=D,
                    transpose=True,
                    queue_num=sup % n_queues,
                    single_packet=single_pkt,
                )
                nc.scalar.copy(xall[:, sup], stg[:])
        last_xt = None
        for sup in range(ngt):
            if body in ("gall", "relay"):
                xt = xall[:, sup]
            elif body == "gfree":
                # fire-and-forget gather into side buffers nothing reads:
                # separates gather *execution* cost from *dependency* cost.
                xt = xts[sup % XB]
                side = xpool.tile([128, 4, n_idx], mdt, tag="side")
                nc.gpsimd.dma_gather(
                    out_ap=side[:],
                    in_ap=emb_h.ap()[:],
                    idxs_ap=idx_sb[:, sup * idx_cols_per_gather:
                                   (sup + 1) * idx_cols_per_gather],
                    num_idxs=n_idx,
                    num_idxs_reg=n_idx,
                    elem_size=D,
                    transpose=True,
                    queue_num=sup % n_queues,
                    single_packet=single_pkt,
                )
            elif body in ("nogather", "mmonly", "randx"):
                xt = xts[sup % XB]
            elif sup % gather_stride != 0 and last_xt is not None:
                xt = last_xt
            else:
                xt = xpool.tile([128, 4, n_idx], mdt, tag="xt")
                nc.gpsimd.dma_gather(
                    out_ap=xt[:],
                    in_ap=emb_h.ap()[:],
                    idxs_ap=idx_sb[:, sup * idx_cols_per_gather:
                                   (sup + 1) * idx_cols_per_gather],
                    num_idxs=n_idx,
                    num_idxs_reg=n_idx,
                    elem_size=D,
                    transpose=True,
                    queue_num=sup % n_queues,
                    single_packet=single_pkt,
                )
                last_xt = xt
            if body == "gonly":
                continue
            xv = xt.rearrange("p c (s l) -> p c s l", s=spg)
            for k_idx, K in enumerate(KS):
                lout = L - K + 1
                for m in range(2):
                    ps = pspool.tile([128, 4, lout], f32, tag="ps",
                                     name=f"ps_{sup}_{k_idx}_{m}")
                    n_mm = 4 * K
                    mm = 0
                    for c in range(4):
                        for j in range(K):
                            w_ap = (w_sb[:, 0, 0, 0, :] if body == "samew"
                                    else w_sb[:, SLOT_BASE[k_idx] + j, c, m, :])
                            nc.tensor.matmul(
                                ps[:],
                                w_ap,
                                xv[:, c, 0:4, j:j + lout],
                                start=(mm == 0),
                                stop=(mm == n_mm - 1),
                            )
                            mm += 1
                    tile6 = k_idx * 2 + m
                    if body not in ("noreduce", "mmonly"):
                        nc.vector.reduce_max(
                            out_sb[:, tile6, sup * 4:sup * 4 + 4],
                            ps[:],
                            axis=mybir.AxisListType.X,
                        )

        if loop_cm is not None:
            loop_cm.__exit__(None, None, None)
        # bias + relu on [f(partition), sample] layout, then PE-transpose so
        # the final DMA writes contiguous [sample, 768] rows.
        for tile6 in range(6):
            nc.vector.tensor_scalar(
                out_sb[:, tile6, :], out_sb[:, tile6, :],
                bias_sb[:, tile6:tile6 + 1], 0.0,
                op0=mybir.AluOpType.add, op1=mybir.AluOpType.max,
            )
            tp = tppool.tile([nsamp, 128], f32, tag="tp")
            nc.tensor.transpose(tp[:], out_sb[:, tile6, :], ident[:])
            nc.vector.tensor_copy(
                out_t[:, tile6 * 128:(tile6 + 1) * 128], tp[:])
        nc.sync.dma_start(out=out_h.ap()[:], in_=out_t[:])

    if "coarse" in flags:
        _coarsen_pe_sem(nc)
    if "nodw" in flags:
        # timing-experiment only: strip DMA-completion sem waits off PE
        # instructions (breaks correctness; tests the cost of PE-side
        # DMA-sem waits).
        for b in nc.m.functions[0].blocks:
            for inst in b.instructions:
                if getattr(inst, "engine", None) != mybir.EngineType.PE:
                    continue
                si = inst.sync_info
                if si is None or not si.on_wait:
                    continue
                kept = [w for w in si.on_wait
                        if not (w.ant_name or "").startswith("DMASW")]
                if len(kept) != len(si.on_wait):
                    inst.sync_info = mybir.SyncInfo(
                        on_wait=kept, on_update=si.on_update)
    nc.compile()
    return nc


def _coarsen_pe_sem(nc):
    """Keep S[PE]++1 only on group-final matmuls (stop=True) and non-matmul
    PE instructions; remap every S[PE] wait/update constant accordingly.

    Sound because the PE engine queue is strict FIFO: the group-final MM
    completing implies all earlier MMs completed.  All existing waits sit at
    group boundaries; any that don't are rounded UP (stronger sync).
    """
    fn = nc.m.functions[0]
    pe_name = None
    # locate the PE tick semaphore name (updated by Matmults with ++1)
    for b in fn.blocks:
        for inst in b.instructions:
            if type(inst).__name__ != "InstMatmult":
                continue
            si = inst.sync_info
            if si is None:
                continue
            for u in (si.on_update or []):
                nm = u.ant_name or ""
                if nm.startswith("PE_") and u.update_mode == "sem-inc":
                    pe_name = nm
                    break
            if pe_name:
                break
        if pe_name:
            break
    if pe_name is None:
        return

    # Pass 1 over the loop-body (largest) block: build old->new count map and
    # strip updates from non-final matmuls.
    blocks = list(fn.blocks)
    body = max(blocks, key=lambda b: len(list(b.instructions)))
    old2new = {0: 0}
    old_c = new_c = 0
    for inst in body.instructions:
        si = inst.sync_info
        if si is None:
            continue
        ups = si.on_update or []
        has_pe = [u for u in ups if (u.ant_name or "") == pe_name]
        if not has_pe:
            continue
        old_c += sum(u.update_value for u in has_pe)
        drop = (type(inst).__name__ == "InstMatmult"
                and inst.stop_tensor_calc is False)
        if drop:
            kept = [u for u in ups if (u.ant_name or "") != pe_name]
            inst.sync_info = mybir.SyncInfo(on_wait=si.on_wait, on_update=kept)
        else:
            new_c += sum(u.update_value for u in has_pe)
        old2new[old_c] = new_c
    total_old, total_new = old_c, new_c

    def remap(v):
        if v in old2new:
            return old2new[v]
        if v > total_old:
            # counts continuing past the loop body (tail PE instructions,
            # which all retain their updates)
            return total_new + (v - total_old)
        # round UP to the next retained boundary (stronger sync, still sound)
        cand = [o for o in old2new if o >= v]
        return old2new[min(cand)]

    # Pass 2 over all blocks: remap waits and bulk update constants.
    for b in blocks:
        for inst in b.instructions:
            si = inst.sync_info
            if si is None:
                continue
            changed = False
            new_waits = []
            for w in (si.on_wait or []):
                if (w.ant_name or "") == pe_name and w.wait_value:
                    new_waits.append(mybir.SyncWait(
                        sync_type=w.sync_type, id=w.id, ant_name=w.ant_name,
                        wait_mode=w.wait_mode, wait_value=remap(w.wait_value),
                        wait_reg=w.wait_reg))
                    changed = True
                else:
                    new_waits.append(w)
            new_ups = []
            for u in (si.on_update or []):
                if ((u.ant_name or "") == pe_name and b is not body
                        and u.update_value and u.update_value > 1):
                    new_ups.append(mybir.SyncUpdate(
                        sync_type=u.sync_type, id=u.id, ant_name=u.ant_name,
                        update_mode=u.update_mode,
                        update_value=remap(u.update_value),
                        update_reg=u.update_reg))
                    changed = True
                else:
                    new_ups.append(u)
            if changed:
                inst.sync_info = mybir.SyncInfo(on_wait=new_waits,
                                                on_update=new_ups)


def prep_inputs(text, embed, w3, b3, w4, b4, w5, b5, nsamp=NSAMP, spg=SPG,
                n_cores=N_CORES, dt_np=None):
    """Host-side marshaling: shard text, wrap gather indices, 16-bit-quantize
    and retile the weights/embedding."""
    if dt_np is None:
        import ml_dtypes
        dt_np = ml_dtypes.bfloat16 if "bf" in BEST else np.float16
    text = np.ascontiguousarray(np.asarray(text).reshape(B * S, L))
    assert text.max() < V and text.min() >= 0
    emb16 = np.ascontiguousarray(np.asarray(embed, dtype=np.float32)
                                 .astype(dt_np))

    wst = np.zeros((128, N_SLOTS, 4, 2, 128), dt_np)
    for k_idx, w in enumerate((w3, w4, w5)):
        w = np.asarray(w, dtype=np.float32)
        for j in range(KS[k_idx]):
            # wst[dd, slot, c, m, ff] = w[m*128+ff, c*128+dd, j]
            wj = w[:, :, j].reshape(2, 128, 4, 128)      # [m, ff, c, dd]
            wst[:, SLOT_BASE[k_idx] + j] = wj.transpose(3, 2, 0, 1)
    wst = np.ascontiguousarray(wst)

    bias = np.zeros((128, 6), np.float32)
    for k_idx, b in enumerate((b3, b4, b5)):
        bias[:, 2 * k_idx:2 * k_idx + 2] = \
            np.asarray(b, dtype=np.float32).reshape(2, 128).T
    bias = np.ascontiguousarray(bias)

    ngt = nsamp // spg
    in_maps = []
    for r in range(n_cores):
        tcore = text[r * nsamp:(r + 1) * nsamp].astype(np.int16)
        # token i of gather tile t -> partition i%16, column t*(spg*L/16)+i//16;
        # the 16-row block must be replicated to all 128 partitions (each of
        # the 8 gpsimd sub-cores reads its own 16-partition stripe).
        a = tcore.reshape(ngt, spg * L // 16, 16)         # [t, c, p]
        idx = np.tile(a.transpose(2, 0, 1).reshape(16, -1), (8, 1))
        in_maps.append({
            "emb": emb16,
            "idx": np.ascontiguousarray(idx),
            "wst": wst,
            "bias": bias,
        })
    return in_maps


_CACHE = {}

# Production variant: all-upfront gathers into one SBUF-resident x tile,
# 4 SWDGE queues, coarsened PE tick semaphore (group-final increments only).
BEST = "gall+q4+bf"


def kernel(text, embed, w3, b3, w4, b4, w5, b5):
    if "nc" not in _CACHE:
        _CACHE["nc"] = build_nc(mode=BEST)
    nc = _CACHE["nc"]
    in_maps = prep_inputs(text, embed, w3, b3, w4, b4, w5, b5)
    res = run_bass_kernel_spmd(nc, in_maps, list(range(N_CORES)))
    out = np.concatenate([res.results[r]["out"] for r in range(N_CORES)],
                         axis=0)
    return out.reshape(B, S, 3 * F).astype(np.float32)

